# revision 5
# baseline (speedup 1.0000x reference)
"""Trainium2 Bass kernel for nn_ArmModel (7-DOF arm forward dynamics, RK4).

Self-contained: takes FULL inputs, shards batch 8192 across 8 NeuronCores
(pure data parallel), runs a Bass/Tile kernel per core, gathers FULL output.

Layout: per core batch 1024 = 128 partitions x 8 free lanes. Every per-lane
scalar is a "slot" = 8 contiguous f32 in the free dim. Ops are batched into
single DVE instructions via strided/broadcast access patterns. All model
parameters (M, A, L, gravity, ftip) are folded into compile-time constants.
"""
import dataclasses
import sys

import numpy as np

sys.path.insert(0, '/opt/trn_rl_repo')

DOF = 7
DT = 0.1
ACTION_RANGE = 50.0
MAX_VEL = 20.0
P = 128
T = 8
NCORE = 1024
NSHARD = 8
MAGIC = 12582912.0
TWO_PI = float(2 * np.pi)


def _skew(v):
    x, y, z = v
    return np.array([[0., -z, y], [z, 0., -x], [-y, x, 0.]], np.float64)


def host_constants(M, A, L, gravity, ftip):
    n = DOF
    M = np.asarray(M, np.float64)
    A = np.asarray(A, np.float64)
    L = np.asarray(L, np.float64)
    gravity = np.asarray(gravity, np.float64)
    ftip = np.asarray(ftip, np.float64)
    G = L @ np.swapaxes(L, -1, -2)
    invM = np.linalg.inv(M)
    C = {'G': G, 'A': A, 'g': gravity}
    nw = np.linalg.norm(A[:, :3], axis=1)
    assert np.all(nw > 1e-8), "prismatic joints unsupported"
    C['nw'] = nw
    ROT0, ROTA, ROTB = [], [], []
    P0, PA, PB, PQ, PG = [], [], [], [], []
    for i in range(n):
        Aw, Av = A[i, :3], A[i, 3:]
        KA = _skew(-Aw)
        KA2 = KA @ KA
        Rm = invM[i][:3, :3]
        pm = invM[i][:3, 3]
        ROT0.append(Rm)
        ROTA.append(KA @ Rm)
        ROTB.append(KA2 @ Rm)
        P0.append(pm)
        PA.append(KA @ pm)
        PB.append(KA2 @ pm - KA @ Av)
        PQ.append(-Av)
        PG.append(-KA2 @ Av)
    C.update(ROT0=np.stack(ROT0), ROTA=np.stack(ROTA), ROTB=np.stack(ROTB),
             P0=np.stack(P0), PA=np.stack(PA), PB=np.stack(PB),
             PQ=np.stack(PQ), PG=np.stack(PG))
    FR0, FRA, FRB = [], [], []
    FP0, FPA, FPB, FPQ, FPG = [], [], [], [], []
    for i in range(n):
        Aw, Av = A[i, :3], A[i, 3:]
        KA = _skew(-Aw)
        KA2 = KA @ KA
        Rm, pm = M[i][:3, :3], M[i][:3, 3]
        R0, RA, RB = Rm, -Rm @ KA, Rm @ KA2
        p0, pa = pm, np.zeros(3)
        pb = -Rm @ (KA @ Av)
        pq = Rm @ Av
        pg = Rm @ (KA2 @ Av)
        if i == n - 1:
            Rm7, pm7 = M[n][:3, :3], M[n][:3, 3]
            p0 = p0 + R0 @ pm7
            pa = pa + RA @ pm7
            pb = pb + RB @ pm7
            R0, RA, RB = R0 @ Rm7, RA @ Rm7, RB @ Rm7
        FR0.append(R0); FRA.append(RA); FRB.append(RB)
        FP0.append(p0); FPA.append(pa); FPB.append(pb)
        FPQ.append(pq); FPG.append(pg)
    C.update(FR0=np.stack(FR0), FRA=np.stack(FRA), FRB=np.stack(FRB),
             FP0=np.stack(FP0), FPA=np.stack(FPA), FPB=np.stack(FPB),
             FPQ=np.stack(FPQ), FPG=np.stack(FPG))
    C['Sw'] = np.stack([_skew(-A[i, :3]) for i in range(n)])
    C['Sv'] = np.stack([_skew(-A[i, 3:]) for i in range(n)])
    iM7 = invM[n]
    AdT_end = np.zeros((6, 6))
    AdT_end[:3, :3] = iM7[:3, :3]
    AdT_end[3:, 3:] = iM7[:3, :3]
    AdT_end[3:, :3] = _skew(iM7[:3, 3]) @ iM7[:3, :3]
    C['F_init'] = AdT_end.T @ ftip
    return C


class ConstImage:
    def __init__(self):
        self.vals = []

    def add(self, arr):
        off = len(self.vals)
        self.vals.extend(np.asarray(arr, np.float64).reshape(-1).tolist())
        return off

    def array(self):
        return np.asarray(self.vals, np.float32)


class KB:
    def __init__(self):
        import concourse.bacc as bacc
        import concourse.mybir as mybir
        self.mybir = mybir
        nc = bacc.Bacc("TRN2", target_bir_lowering=False, debug=False)
        self.nc = nc
        t = nc.alloc_sbuf_tensor("const-f32-half-pi", [P, 1], mybir.dt.float32)
        nc.gpsimd.memset(t.ap(), float(np.pi / 2))
        nc.const_aps.aps[(mybir.dt.float32, float(np.pi / 2))] = t.ap()
        nc.all_engine_barrier()
        self.op = {k: getattr(mybir.AluOpType, k) for k in
                   ('mult', 'add', 'subtract', 'max', 'min', 'is_lt', 'is_ge')}
        self.af = mybir.ActivationFunctionType
        self.arenas = {}

    def make_arena(self, pool, name, slots, elem_per_slot=T):
        width = slots * elem_per_slot
        tile = pool.tile([P, width], self.mybir.dt.float32, tag=name)
        self.arenas[name] = (tile[:, :], width, elem_per_slot)

    def raw(self, arena, ap_dims, offset_elems):
        base, width, _ = self.arenas[arena]
        return dataclasses.replace(
            base, ap=self.mybir.VecI64Pair([[width, P]] + list(ap_dims)),
            offset=offset_elems)

    def v(self, arena, off, dims=(), t_stride=1, t_count=T):
        """View: offset in slots; dims = seq of (count, stride_slots);
        t-dim appended."""
        _, _, eps = self.arenas[arena]
        ap = [[st * eps, cnt] for cnt, st in dims] + [[t_stride, t_count]]
        return self.raw(arena, ap, off * eps)

    def cv(self, off, dims=()):
        return self.v('cstT', off, dims)

    def tt(self, out, a, b, op='mult'):
        self.nc.vector.tensor_tensor(out, a, b, self.op[op])

    def ts(self, out, a, s1, op0='mult', s2=None, op1='add'):
        if s2 is None:
            self.nc.vector.tensor_scalar(out, a, float(s1), None,
                                         self.op[op0])
        else:
            self.nc.vector.tensor_scalar(out, a, float(s1), float(s2),
                                         self.op[op0], self.op[op1])

    def stt(self, out, a, imm, b, op0='mult', op1='add'):
        self.nc.vector.scalar_tensor_tensor(out, a, float(imm), b,
                                            self.op[op0], self.op[op1])

    def red(self, out, in_, op='add'):
        self.nc.vector.tensor_reduce(out, in_, self.mybir.AxisListType.X,
                                     self.op[op])

    def act(self, out, in_, func, bias=0.0, scale=1.0):
        self.nc.scalar.activation(out, in_, getattr(self.af, func),
                                  bias=float(bias), scale=float(scale))

    def cp(self, out, in_):
        self.nc.vector.tensor_copy(out, in_)

    def scp(self, out, in_):
        self.nc.scalar.activation(out, in_, self.af.Copy)

    def recip(self, out, in_, scratch):
        self.nc.vector.reciprocal_approx_accurate(out, in_, scratch)

    def memset(self, ap, val):
        self.nc.vector.memset(ap, float(val))


def build(C):
    """Emit + compile the bass program. Returns (nc, const_array)."""
    kb = KB()
    nc = kb.nc
    mybir = kb.mybir
    n = DOF
    v, cv = kb.v, kb.cv
    ci = ConstImage()

    cROT0 = ci.add(C['ROT0']); cROTA = ci.add(C['ROTA']); cROTB = ci.add(C['ROTB'])
    cP0 = ci.add(C['P0']); cPA = ci.add(C['PA']); cPB = ci.add(C['PB'])
    cPQ = ci.add(C['PQ']); cPG = ci.add(C['PG'])
    cA = ci.add(C['A'])
    cSw = ci.add(C['Sw']); cSv = ci.add(C['Sv'])
    cG = ci.add(C['G'])
    cFR0 = ci.add(C['FR0']); cFRA = ci.add(C['FRA']); cFRB = ci.add(C['FRB'])
    cFP0 = ci.add(C['FP0']); cFPA = ci.add(C['FPA']); cFPB = ci.add(C['FPB'])
    cFPQ = ci.add(C['FPQ']); cFPG = ci.add(C['FPG'])
    cNW = ci.add(C['nw']); cINW = ci.add(1.0 / C['nw'])
    cINW2 = ci.add(1.0 / C['nw'] ** 2); cINW3 = ci.add(1.0 / C['nw'] ** 3)
    cFINIT = ci.add(C['F_init']); cNEGG = ci.add(-C['g'])
    n_cst = len(ci.vals)

    state_ext = nc.declare_dram_parameter("state", [NCORE, 14],
                                          mybir.dt.float32, isOutput=False)
    torque_ext = nc.declare_dram_parameter("torque", [NCORE, 7],
                                           mybir.dt.float32, isOutput=False)
    cst_ext = nc.declare_dram_parameter("cst", [P, n_cst],
                                        mybir.dt.float32, isOutput=False)
    out_ext = nc.declare_dram_parameter("out", [NCORE, 16],
                                        mybir.dt.float32, isOutput=True)

    # st arena slots
    Q0, DQ0, TAU, QS, DQS, ACCQ, ACCDQ, QAC, SC2 = \
        0, 7, 14, 21, 28, 35, 42, 49, 56
    # trig arena slots
    tX, tS, tC, tAL, tBE, tGA, tW, tSC = 0, 7, 14, 21, 28, 35, 42, 49
    # sol arena slots
    MM, H, DINV, LFQ, RHS, Y, XS, SCR = 0, 49, 56, 63, 112, 119, 126, 133

    hist_off = [0]
    for i in range(n):
        hist_off.append(hist_off[-1] + (i + 3) * 6)
    HQ = hist_off

    RJ = lambda j: j * 9
    PJ = lambda j: 63 + j * 5
    RJT = lambda j: 98 + j * 9

    from concourse.tile import TileContext
    with TileContext(nc) as tc:
        with tc.tile_pool(name="pool", bufs=1) as pool:
            kb.make_arena(pool, 'cst', n_cst, elem_per_slot=1)
            kb.make_arena(pool, 'cstT', n_cst)
            kb.make_arena(pool, 'io', 21)
            kb.make_arena(pool, 'st', 64)
            kb.make_arena(pool, 'trig', 56)
            kb.make_arena(pool, 'Rp', 161)
            kb.make_arena(pool, 'hist', 294)
            kb.make_arena(pool, 'mv', 280)
            kb.make_arena(pool, 'fb', 48)
            kb.make_arena(pool, 'gp', 324)
            kb.make_arena(pool, 'gmv', 54)
            kb.make_arena(pool, 'ex', 80)
            kb.make_arena(pool, 'sol', 160)
            kb.make_arena(pool, 'fk', 120)
            kb.make_arena(pool, 'outb', 16)

            # ---------------- DMA in + unshuffle ----------------
            io_base, io_w, _ = kb.arenas['io']
            nc.sync.dma_start(
                out=io_base[:, 0:112],
                in_=state_ext.ap().rearrange("(p t) c -> p (t c)", p=P))
            nc.sync.dma_start(
                out=io_base[:, 112:168],
                in_=torque_ext.ap().rearrange("(p t) c -> p (t c)", p=P))
            cst_b, _, _ = kb.arenas['cst']
            nc.sync.dma_start(out=cst_b[:, 0:n_cst], in_=cst_ext.ap())
            kb.cp(kb.raw('cstT', [[1, 8 * n_cst]], 0),
                  kb.raw('cst', [[1, n_cst], [0, T]], 0))
            # (t c) -> slots
            kb.cp(v('st', Q0, ((14, 1),)),
                  kb.raw('io', [[1, 14], [14, T]], 0))
            kb.ts(v('st', TAU, ((7, 1),)),
                  kb.raw('io', [[1, 7], [7, T]], 112), ACTION_RANGE)

            # ---------------- helpers ----------------
            def emit_trig(q_arena, q_slot):
                kb.tt(v('trig', tX, ((7, 1),)), v(q_arena, q_slot, ((7, 1),)),
                      cv(cNW, ((7, 1),)))
                kb.ts(v('trig', tW, ((7, 1),)), v('trig', tX, ((7, 1),)),
                      1.0 / TWO_PI, 'mult', MAGIC, 'add')
                kb.ts(v('trig', tW, ((7, 1),)), v('trig', tW, ((7, 1),)),
                      MAGIC, 'subtract')
                kb.stt(v('trig', tW, ((7, 1),)), v('trig', tW, ((7, 1),)),
                       -TWO_PI, v('trig', tX, ((7, 1),)), 'mult', 'add')
                kb.act(v('trig', tS, ((7, 1),)), v('trig', tW, ((7, 1),)), 'Sin')
                kb.act(v('trig', tSC, ((7, 1),)), v('trig', tW, ((7, 1),)), 'Abs')
                kb.act(v('trig', tC, ((7, 1),)), v('trig', tSC, ((7, 1),)),
                       'Sin', bias=float(np.pi / 2), scale=-1.0)
                kb.tt(v('trig', tAL, ((7, 1),)), v('trig', tS, ((7, 1),)),
                      cv(cINW, ((7, 1),)))
                kb.ts(v('trig', tSC, ((7, 1),)), v('trig', tC, ((7, 1),)),
                      -1.0, 'mult', 1.0, 'add')
                kb.tt(v('trig', tBE, ((7, 1),)), v('trig', tSC, ((7, 1),)),
                      cv(cINW2, ((7, 1),)))
                kb.tt(v('trig', tSC, ((7, 1),)), v('trig', tX, ((7, 1),)),
                      v('trig', tS, ((7, 1),)), 'subtract')
                kb.tt(v('trig', tGA, ((7, 1),)), v('trig', tSC, ((7, 1),)),
                      cv(cINW3, ((7, 1),)))

            def emit_T_build(q_arena, q_slot, rot0, rota, rotb,
                             p0, pa, pb, pq, pg):
                Rv = v('Rp', 0, ((7, 9), (9, 1)))
                al_b = v('trig', tAL, ((7, 1), (9, 0)))
                be_b = v('trig', tBE, ((7, 1), (9, 0)))
                kb.tt(Rv, al_b, cv(rota, ((7, 9), (9, 1))))
                kb.tt(Rv, Rv, cv(rot0, ((7, 9), (9, 1))), 'add')
                tmp9 = v('mv', 0, ((7, 9), (9, 1)))
                kb.tt(tmp9, be_b, cv(rotb, ((7, 9), (9, 1))))
                kb.tt(Rv, Rv, tmp9, 'add')
                pv = v('Rp', 63, ((7, 5), (3, 1)))
                al3 = v('trig', tAL, ((7, 1), (3, 0)))
                be3 = v('trig', tBE, ((7, 1), (3, 0)))
                ga3 = v('trig', tGA, ((7, 1), (3, 0)))
                q3 = v(q_arena, q_slot, ((7, 1), (3, 0)))
                tmp3 = v('mv', 63, ((7, 3), (3, 1)))
                kb.tt(pv, al3, cv(pa, ((7, 3), (3, 1))))
                kb.tt(pv, pv, cv(p0, ((7, 3), (3, 1))), 'add')
                kb.tt(tmp3, be3, cv(pb, ((7, 3), (3, 1))))
                kb.tt(pv, pv, tmp3, 'add')
                kb.tt(tmp3, q3, cv(pq, ((7, 3), (3, 1))))
                kb.tt(pv, pv, tmp3, 'add')
                kb.tt(tmp3, ga3, cv(pg, ((7, 3), (3, 1))))
                kb.tt(pv, pv, tmp3, 'add')
                kb.scp(v('Rp', 63 + 3, ((7, 5), (2, 1))),
                       v('Rp', 63, ((7, 5), (2, 1))))
                for j in range(7):
                    kb.scp(v('Rp', RJT(j), ((9, 1),)),
                           v('Rp', RJ(j), ((3, 1), (3, 3))))

            def adj_matvec(j, src_off, nvec, dst_off):
                """hist[dst] = AdT_j @ hist[src] for nvec contiguous 6-vecs."""
                sv = 2 * nvec
                prod = v('mv', 0, ((sv, 9), (3, 3), (3, 1)))
                kb.tt(prod, v('Rp', RJ(j), ((sv, 0), (3, 3), (3, 1))),
                      v('hist', src_off, ((sv, 3), (3, 0), (3, 1))))
                MVV = sv * 9
                _, mw, me = kb.arenas['mv']
                hb, hw, he = kb.arenas['hist']
                def red_in(first_sv):
                    return kb.raw('mv', [[18 * me, nvec], [3 * me, 3],
                                         [1, T], [me, 3]], first_sv * 9 * me)
                kb.red(kb.raw('hist', [[6 * he, nvec], [he, 3], [1, T]],
                              dst_off * he), red_in(0))
                kb.red(kb.raw('mv', [[3 * me, nvec], [me, 3], [1, T]],
                              MVV * me), red_in(1))
                kb.scp(v('ex', 0, ((nvec, 5), (3, 1))),
                       v('hist', dst_off, ((nvec, 6), (3, 1))))
                kb.scp(v('ex', 3, ((nvec, 5), (2, 1))),
                       v('hist', dst_off, ((nvec, 6), (2, 1))))
                CRP = MVV + 3 * nvec
                kb.tt(v('mv', CRP, ((nvec, 6), (2, 3), (3, 1))),
                      v('Rp', PJ(j) + 1, ((nvec, 0), (2, 1), (3, 1))),
                      v('ex', 2, ((nvec, 5), (2, -1), (3, 1))))
                CRS = CRP + 6 * nvec
                kb.tt(v('mv', CRS, ((nvec, 3), (3, 1))),
                      v('mv', CRP, ((nvec, 6), (3, 1))),
                      v('mv', CRP + 3, ((nvec, 6), (3, 1))), 'subtract')
                kb.tt(v('hist', dst_off + 3, ((nvec, 6), (3, 1))),
                      v('mv', CRS, ((nvec, 3), (3, 1))),
                      v('mv', MVV, ((nvec, 3), (3, 1))), 'add')

            def adjT_matvec(j, nvec):
                """fb[0..nvec) = AdT_j^T @ fb[0..nvec) in place."""
                kb.scp(v('ex', 0, ((nvec, 5), (3, 1))),
                       v('fb', 3, ((nvec, 6), (3, 1))))
                kb.scp(v('ex', 3, ((nvec, 5), (2, 1))),
                       v('fb', 3, ((nvec, 6), (2, 1))))
                kb.tt(v('mv', 0, ((nvec, 6), (2, 3), (3, 1))),
                      v('ex', 1, ((nvec, 5), (2, 1), (3, 1))),
                      v('Rp', PJ(j) + 2, ((nvec, 0), (2, -1), (3, 1))))
                PAIR = 6 * nvec
                CRS = PAIR + 6 * nvec
                kb.tt(v('mv', CRS, ((nvec, 3), (3, 1))),
                      v('mv', 0, ((nvec, 6), (3, 1))),
                      v('mv', 3, ((nvec, 6), (3, 1))), 'subtract')
                kb.tt(v('mv', PAIR, ((nvec, 6), (3, 1))),
                      v('mv', CRS, ((nvec, 3), (3, 1))),
                      v('fb', 0, ((nvec, 6), (3, 1))), 'add')
                kb.scp(v('mv', PAIR + 3, ((nvec, 6), (3, 1))),
                       v('fb', 3, ((nvec, 6), (3, 1))))
                sv = 2 * nvec
                PR = CRS + 3 * nvec
                kb.tt(v('mv', PR, ((sv, 9), (3, 3), (3, 1))),
                      v('Rp', RJT(j), ((sv, 0), (3, 3), (3, 1))),
                      v('mv', PAIR, ((sv, 3), (3, 0), (3, 1))))
                _, mw, me = kb.arenas['mv']
                fbb, fbw, fbe = kb.arenas['fb']
                def red_in(first_sv):
                    return kb.raw('mv', [[18 * me, nvec], [3 * me, 3],
                                         [1, T], [me, 3]],
                                  (PR + first_sv * 9) * me)
                kb.red(kb.raw('fb', [[6 * fbe, nvec], [fbe, 3], [1, T]], 0),
                       red_in(0))
                kb.red(kb.raw('fb', [[6 * fbe, nvec], [fbe, 3], [1, T]],
                              3 * fbe), red_in(1))

            def g_matvec(i, nvec):
                """gmv[0..nvec) = G_i @ hist[HQ[i]..] (nvec 6-vecs)."""
                kb.tt(v('gp', 0, ((nvec, 36), (6, 6), (6, 1))),
                      cv(cG + 36 * i, ((nvec, 0), (6, 6), (6, 1))),
                      v('hist', HQ[i], ((nvec, 6), (6, 0), (6, 1))))
                _, gw, ge = kb.arenas['gp']
                _, vw, ve = kb.arenas['gmv']
                kb.red(kb.raw('gmv', [[6 * ve, nvec], [ve, 6], [1, T]], 0),
                       kb.raw('gp', [[36 * ge, nvec], [6 * ge, 6],
                                     [1, T], [ge, 6]], 0))

            # ---------------- derivs ----------------
            def emit_derivs(q_arena, q_slot, dq_slot):
                emit_trig(q_arena, q_slot)
                emit_T_build(q_arena, q_slot, cROT0, cROTA, cROTB,
                             cP0, cPA, cPB, cPQ, cPG)
                # forward
                for i in range(n):
                    if i == 0:
                        kb.tt(v('mv', 0, ((3, 3), (3, 1))),
                              v('Rp', RJ(0), ((3, 3), (3, 1))),
                              cv(cNEGG, ((3, 0), (3, 1))))
                        _, mw, me = kb.arenas['mv']
                        _, hw, he = kb.arenas['hist']
                        kb.red(kb.raw('hist', [[he, 3], [1, T]],
                                      (HQ[0] + 3) * he),
                               kb.raw('mv', [[3 * me, 3], [1, T], [me, 3]], 0))
                        kb.memset(v('hist', HQ[0], ((3, 1),)), 0.0)
                        kb.tt(v('hist', HQ[0] + 6, ((6, 1),)),
                              v('st', dq_slot + 0, ((6, 0),)),
                              cv(cA, ((6, 1),)))
                    else:
                        adj_matvec(i, HQ[i - 1], i + 2, HQ[i])
                        kb.tt(v('mv', 240, ((6, 1),)),
                              v('st', dq_slot + i, ((6, 0),)),
                              cv(cA + 6 * i, ((6, 1),)))
                        kb.tt(v('hist', HQ[i] + 6, ((6, 1),)),
                              v('hist', HQ[i] + 6, ((6, 1),)),
                              v('mv', 240, ((6, 1),)), 'add')
                    # ad-term
                    kb.tt(v('mv', 0, ((2, 9), (3, 3), (3, 1))),
                          cv(cSw + 9 * i, ((2, 0), (3, 3), (3, 1))),
                          v('hist', HQ[i] + 6, ((2, 3), (3, 0), (3, 1))))
                    _, mw, me = kb.arenas['mv']
                    kb.red(kb.raw('mv', [[3 * me, 2], [me, 3], [1, T]],
                                  18 * me),
                           kb.raw('mv', [[9 * me, 2], [3 * me, 3],
                                         [1, T], [me, 3]], 0))
                    kb.tt(v('mv', 24, ((3, 3), (3, 1))),
                          cv(cSv + 9 * i, ((3, 3), (3, 1))),
                          v('hist', HQ[i] + 6, ((3, 0), (3, 1))))
                    kb.red(kb.raw('mv', [[me, 3], [1, T]], 33 * me),
                           kb.raw('mv', [[3 * me, 3], [1, T], [me, 3]],
                                  24 * me))
                    kb.tt(v('mv', 21, ((3, 1),)), v('mv', 21, ((3, 1),)),
                          v('mv', 33, ((3, 1),)), 'add')
                    kb.tt(v('mv', 24, ((6, 1),)), v('mv', 18, ((6, 1),)),
                          v('st', dq_slot + i, ((6, 0),)))
                    kb.tt(v('hist', HQ[i], ((6, 1),)),
                          v('hist', HQ[i], ((6, 1),)),
                          v('mv', 24, ((6, 1),)), 'add')
                    # X^i = A_i
                    kb.scp(v('hist', HQ[i] + 12 + 6 * i, ((6, 1),)),
                           cv(cA + 6 * i, ((6, 1),)))
                # backward
                for i in range(n - 1, -1, -1):
                    if i == n - 1:
                        kb.scp(v('fb', 0, ((6, 1),)), cv(cFINIT, ((6, 1),)))
                        kb.memset(v('fb', 6, ((42, 1),)), 0.0)
                    else:
                        adjT_matvec(i + 1, i + 2)
                    g_matvec(i, i + 3)
                    # bias adT term
                    kb.scp(v('ex', 40, ((2, 5), (3, 1))),
                           v('hist', HQ[i] + 6, ((2, 3), (3, 1))))
                    kb.scp(v('ex', 43, ((2, 5), (2, 1))),
                           v('hist', HQ[i] + 6, ((2, 3), (2, 1))))
                    kb.scp(v('ex', 50, ((2, 5), (3, 1))),
                           v('gmv', 6, ((2, 3), (3, 1))))
                    kb.scp(v('ex', 53, ((2, 5), (2, 1))),
                           v('gmv', 6, ((2, 3), (2, 1))))
                    kb.tt(v('mv', 86, ((2, 6), (2, 3), (3, 1))),
                          v('ex', 41, ((2, 5), (2, 1), (3, 1))),
                          v('ex', 52, ((2, 5), (2, -1), (3, 1))))
                    kb.tt(v('mv', 80, ((2, 3), (3, 1))),
                          v('mv', 86, ((2, 6), (3, 1))),
                          v('mv', 89, ((2, 6), (3, 1))), 'subtract')
                    kb.tt(v('mv', 100, ((3, 1),)), v('mv', 80, ((3, 1),)),
                          v('mv', 83, ((3, 1),)), 'add')
                    kb.tt(v('mv', 86, ((1, 6), (2, 3), (3, 1))),
                          v('ex', 41, ((1, 5), (2, 1), (3, 1))),
                          v('ex', 57, ((1, 5), (2, -1), (3, 1))))
                    kb.tt(v('mv', 103, ((3, 1),)), v('mv', 86, ((3, 1),)),
                          v('mv', 89, ((3, 1),)), 'subtract')
                    kb.tt(v('fb', 0, ((6, 1),)), v('fb', 0, ((6, 1),)),
                          v('gmv', 0, ((6, 1),)), 'add')
                    kb.tt(v('fb', 0, ((6, 1),)), v('fb', 0, ((6, 1),)),
                          v('mv', 100, ((6, 1),)), 'add')
                    kb.tt(v('fb', 6, ((6 * (i + 1), 1),)),
                          v('fb', 6, ((6 * (i + 1), 1),)),
                          v('gmv', 12, ((6 * (i + 1), 1),)), 'add')
                    # dots
                    kb.tt(v('mv', 0, ((i + 2, 6), (6, 1))),
                          v('fb', 0, ((i + 2, 6), (6, 1))),
                          cv(cA + 6 * i, ((i + 2, 0), (6, 1))))
                    _, mw, me = kb.arenas['mv']
                    _, sw_, se = kb.arenas['sol']
                    kb.red(kb.raw('sol', [[1, T]], (H + i) * se),
                           kb.raw('mv', [[1, T], [me, 6]], 0))
                    kb.red(kb.raw('sol', [[se, i + 1], [1, T]],
                                  (MM + 7 * i) * se),
                           kb.raw('mv', [[6 * me, i + 1], [1, T], [me, 6]],
                                  6 * me))
                # mirror upper triangle
                _, sw_, se = kb.arenas['sol']
                for jj in range(n - 1):
                    cnt = n - 1 - jj
                    kb.scp(v('sol', MM + 7 * jj + jj + 1, ((cnt, 1),)),
                           kb.raw('sol', [[7 * se, cnt], [1, T]],
                                  (MM + 7 * (jj + 1) + jj) * se))
                # rhs = tau - h
                kb.tt(v('sol', RHS, ((7, 1),)), v('st', TAU, ((7, 1),)),
                      v('sol', H, ((7, 1),)), 'subtract')
                # LDLt
                for k in range(n):
                    kb.recip(v('sol', DINV + k, ()),
                             v('sol', MM + 7 * k + k, ()),
                             v('sol', SCR, ()))
                    r = n - 1 - k
                    if r > 0:
                        mcol = kb.raw('sol', [[7 * se, r], [1, T]],
                                      (MM + 7 * (k + 1) + k) * se)
                        lcol = kb.raw('sol', [[7 * se, r], [1, T]],
                                      (LFQ + 7 * (k + 1) + k) * se)
                        kb.tt(lcol, mcol, v('sol', DINV + k, ((r, 0),)))
                        kb.tt(v('mv', 0, ((r, r), (r, 1))),
                              kb.raw('sol', [[7 * se, r], [0, r], [1, T]],
                                     (LFQ + 7 * (k + 1) + k) * se),
                              kb.raw('sol', [[0, r], [se, r], [1, T]],
                                     (MM + 7 * k + k + 1) * se))
                        msub = kb.raw('sol', [[7 * se, r], [se, r], [1, T]],
                                      (MM + 7 * (k + 1) + k + 1) * se)
                        kb.tt(msub, msub, v('mv', 0, ((r, r), (r, 1))),
                              'subtract')
                kb.scp(v('sol', Y, ((7, 1),)), v('sol', RHS, ((7, 1),)))
                for k in range(n - 1):
                    r = n - 1 - k
                    lcol = kb.raw('sol', [[7 * se, r], [1, T]],
                                  (LFQ + 7 * (k + 1) + k) * se)
                    kb.tt(v('mv', 0, ((r, 1),)), lcol,
                          v('sol', Y + k, ((r, 0),)))
                    kb.tt(v('sol', Y + k + 1, ((r, 1),)),
                          v('sol', Y + k + 1, ((r, 1),)),
                          v('mv', 0, ((r, 1),)), 'subtract')
                kb.tt(v('sol', XS, ((7, 1),)), v('sol', Y, ((7, 1),)),
                      v('sol', DINV, ((7, 1),)))
                for k in range(n - 1, 0, -1):
                    kb.tt(v('mv', 0, ((k, 1),)),
                          v('sol', LFQ + 7 * k, ((k, 1),)),
                          v('sol', XS + k, ((k, 0),)))
                    kb.tt(v('sol', XS, ((k, 1),)), v('sol', XS, ((k, 1),)),
                          v('mv', 0, ((k, 1),)), 'subtract')
                kb.scp(v('st', QAC, ((7, 1),)), v('sol', XS, ((7, 1),)))

            # ---------------- RK4 ----------------
            HDT = 0.5 * DT
            emit_derivs('st', Q0, DQ0)
            kb.scp(v('st', ACCDQ, ((7, 1),)), v('st', QAC, ((7, 1),)))
            kb.scp(v('st', ACCQ, ((7, 1),)), v('st', DQ0, ((7, 1),)))
            kb.stt(v('st', QS, ((7, 1),)), v('st', DQ0, ((7, 1),)), HDT,
                   v('st', Q0, ((7, 1),)))
            kb.stt(v('st', DQS, ((7, 1),)), v('st', QAC, ((7, 1),)), HDT,
                   v('st', DQ0, ((7, 1),)))
            emit_derivs('st', QS, DQS)
            kb.stt(v('st', ACCDQ, ((7, 1),)), v('st', QAC, ((7, 1),)), 2.0,
                   v('st', ACCDQ, ((7, 1),)))
            kb.stt(v('st', ACCQ, ((7, 1),)), v('st', DQS, ((7, 1),)), 2.0,
                   v('st', ACCQ, ((7, 1),)))
            kb.stt(v('st', QS, ((7, 1),)), v('st', DQS, ((7, 1),)), HDT,
                   v('st', Q0, ((7, 1),)))
            kb.stt(v('st', DQS, ((7, 1),)), v('st', QAC, ((7, 1),)), HDT,
                   v('st', DQ0, ((7, 1),)))
            emit_derivs('st', QS, DQS)
            kb.stt(v('st', ACCDQ, ((7, 1),)), v('st', QAC, ((7, 1),)), 2.0,
                   v('st', ACCDQ, ((7, 1),)))
            kb.stt(v('st', ACCQ, ((7, 1),)), v('st', DQS, ((7, 1),)), 2.0,
                   v('st', ACCQ, ((7, 1),)))
            kb.stt(v('st', QS, ((7, 1),)), v('st', DQS, ((7, 1),)), DT,
                   v('st', Q0, ((7, 1),)))
            kb.stt(v('st', DQS, ((7, 1),)), v('st', QAC, ((7, 1),)), DT,
                   v('st', DQ0, ((7, 1),)))
            emit_derivs('st', QS, DQS)
            kb.tt(v('st', ACCDQ, ((7, 1),)), v('st', ACCDQ, ((7, 1),)),
                  v('st', QAC, ((7, 1),)), 'add')
            kb.tt(v('st', ACCQ, ((7, 1),)), v('st', ACCQ, ((7, 1),)),
                  v('st', DQS, ((7, 1),)), 'add')
            # qf -> outb[0:7], dqf -> outb[7:14]
            kb.stt(v('outb', 0, ((7, 1),)), v('st', ACCQ, ((7, 1),)),
                   DT / 6.0, v('st', Q0, ((7, 1),)))
            kb.stt(v('outb', 7, ((7, 1),)), v('st', ACCDQ, ((7, 1),)),
                   DT / 6.0, v('st', DQ0, ((7, 1),)))
            # wrap q to [-pi, pi)
            kb.ts(v('st', SC2, ((7, 1),)), v('outb', 0, ((7, 1),)),
                  1.0 / TWO_PI, 'mult', MAGIC, 'add')
            kb.ts(v('st', SC2, ((7, 1),)), v('st', SC2, ((7, 1),)),
                  MAGIC, 'subtract')
            kb.stt(v('outb', 0, ((7, 1),)), v('st', SC2, ((7, 1),)),
                   -TWO_PI, v('outb', 0, ((7, 1),)), 'mult', 'add')
            # clip dq
            kb.ts(v('outb', 7, ((7, 1),)), v('outb', 7, ((7, 1),)),
                  MAX_VEL, 'min', -MAX_VEL, 'max')
            # ---------------- FK ----------------
            emit_trig('outb', 0)
            emit_T_build('outb', 0, cFR0, cFRA, cFRB,
                         cFP0, cFPA, cFPB, cFPQ, cFPG)
            # chain, keep top 2 rows: TA=fk[0:6] rot(2x3), Tp=fk[12:14]
            TA, TB, TPA, TPB, PRD = 0, 6, 12, 14, 20
            kb.cp(v('fk', TA, ((6, 1),)), v('Rp', RJ(0), ((6, 1),)))
            kb.cp(v('fk', TPA, ((2, 1),)), v('Rp', PJ(0), ((2, 1),)))
            ta, tb, tpa, tpb = TA, TB, TPA, TPB
            for k in range(1, n):
                # rot product: prd[r2][d3][c3] = Trot[r][c] * E[c][d]
                kb.tt(v('fk', PRD, ((2, 9), (3, 3), (3, 1))),
                      v('fk', ta, ((2, 3), (3, 0), (3, 1))),
                      v('Rp', RJT(k), ((2, 0), (3, 3), (3, 1))))
                _, fw, fe = kb.arenas['fk']
                kb.red(kb.raw('fk', [[3 * fe, 2], [fe, 3], [1, T]], tb * fe),
                       kb.raw('fk', [[9 * fe, 2], [3 * fe, 3], [1, T], [fe, 3]],
                              PRD * fe))
                # trans: prd2[r2][c3] = Trot[r][c] * Ep[c] ; reduce + Tp
                kb.tt(v('fk', PRD + 18, ((2, 3), (3, 1))),
                      v('fk', ta, ((2, 3), (3, 1))),
                      v('Rp', PJ(k), ((2, 0), (3, 1))))
                kb.red(kb.raw('fk', [[fe, 2], [1, T]], (PRD + 24) * fe),
                       kb.raw('fk', [[3 * fe, 2], [1, T], [fe, 3]],
                              (PRD + 18) * fe))
                kb.tt(v('fk', tpb, ((2, 1),)), v('fk', PRD + 24, ((2, 1),)),
                      v('fk', tpa, ((2, 1),)), 'add')
                ta, tb = tb, ta
                tpa, tpb = tpb, tpa
            kb.cp(v('outb', 14, ((2, 1),)), v('fk', tpa, ((2, 1),)))
            # ---------------- output shuffle + DMA ----------------
            kb.cp(kb.raw('io', [[1, 16], [16, T]], 0),
                  v('outb', 0, ((16, 1),)))
            nc.sync.dma_start(
                out=out_ext.ap().rearrange("(p t) c -> p (t c)", p=P),
                in_=io_base[:, 0:128])

    nc.compile()
    return nc, ci.array()


# ----------------------------------------------------------------------------
# runner
# ----------------------------------------------------------------------------
_CACHE = {}


def _inject_ntff_hook():
    import types
    import antenv
    if 'antenv.axon_hooks' in sys.modules:
        return
    hooks_mod = types.ModuleType('antenv.axon_hooks')
    state = {'h': None}
    hooks_mod.set_axon_ntff_profile_hook = lambda h: state.update(h=h)
    hooks_mod.get_axon_ntff_profile_hook = lambda: state['h']
    sys.modules['antenv.axon_hooks'] = hooks_mod
    antenv.axon_hooks = hooks_mod
    try:
        from trn_agent_boot.trn_boot import _ntff_profile_via_ctypes
        state['h'] = _ntff_profile_via_ctypes('/opt/axon/libaxon_pjrt.so')
    except Exception:
        pass


def get_compiled(M, A, L, gravity, ftip):
    key = (np.asarray(M).tobytes(), np.asarray(A).tobytes(),
           np.asarray(L).tobytes(), np.asarray(gravity).tobytes(),
           np.asarray(ftip).tobytes())
    if key not in _CACHE:
        C = host_constants(M, A, L, gravity, ftip)
        _CACHE[key] = build(C)
    return _CACHE[key]


def run(state, torque, M, A, L, gravity, ftip, trace=False):
    _inject_ntff_hook()
    from concourse.bass_utils import run_bass_kernel_spmd
    nc, cst = get_compiled(M, A, L, gravity, ftip)
    state = np.ascontiguousarray(np.asarray(state, np.float32))
    torque = np.ascontiguousarray(np.asarray(torque, np.float32))
    B = state.shape[0]
    assert B == NCORE * NSHARD
    cst_arr = np.ascontiguousarray(
        np.broadcast_to(cst[None, :], (P, cst.shape[0])))
    in_maps = []
    for s in range(NSHARD):
        sl = slice(s * NCORE, (s + 1) * NCORE)
        in_maps.append({"state": state[sl], "torque": torque[sl],
                        "cst": cst_arr})
    res = run_bass_kernel_spmd(nc, in_maps, list(range(NSHARD)), trace=trace)
    outs = np.concatenate([res.results[s]["out"] for s in range(NSHARD)], 0)
    s_out = outs[:, 0:14]
    ee = outs[:, 14:16]
    return (s_out, ee), res


def kernel(state, torque, M, A, L, gravity, ftip):
    (s_out, ee), _ = run(state, torque, M, A, L, gravity, ftip)
    return s_out, ee
